# revision 1
# baseline (speedup 1.0000x reference)
"""Deformable attention Bass kernel for Trainium2, sharded over 8 NeuronCores.

Problem: nn_DeformableAttention (Q=40000 queries, C=256, 2 levels of 128x256
feature maps, 1 head, 4 points/level, bilinear grid-sample w/ zero padding).

Strategy (per sharding hint): shard queries across the 8 cores (5000 each,
padded to 5120 = 40 tiles of 128); value feature maps + linear weights are
replicated. Per 128-query tile:
  - PE computes the offset/attention linears from a host-pre-transposed query
  - DVE computes sample coords, clamped corner indices and bilinear "hat"
    weights (position-based weights reproduce zero-padding semantics exactly)
  - one indirect DMA gather per level fetches the 16 corner rows per query
  - 32 fused scalar_tensor_tensor MACs accumulate the weighted rows
"""
import sys
import os

sys.path.insert(0, '/opt/trn_rl_repo')

import numpy as np

import concourse.bass as bass
import concourse.mybir as mybir
from concourse.bass import ts
from concourse.tile import TileContext

F32 = mybir.dt.float32
I32 = mybir.dt.int32

N_CORES = 8
H, W = 128, 256
C = 256
NV = H * W          # rows per level feature map
QTOT = 40000
QPC = QTOT // N_CORES       # 5000 queries per core
P = 128                     # partition/tile size
NT_FULL = (QPC + P - 1) // P  # 40 tiles (last padded)
QPAD = NT_FULL * P          # 5120

NCOL_CONST = 4 + 24 + 16 + 32   # smat, b_all, cmax, cornoff

_WAIT_OP_FROM_MODE = {
    "sem-ge-imm": "sem-ge",
    "sem-eq-imm": "sem-eq",
    "sem-ge": "sem-ge",
    "sem-eq": "sem-eq",
}


def _split_multiwait_noctrl(nc, max_waits=1):
    """This walrus build rejects >1 sync-wait per instruction ("Too many sync
    wait commands"). Hoist extra waits onto standalone single-wait
    EventSemaphore instructions placed immediately before, on the same engine
    (program order on the engine queue preserves semantics)."""
    import bass_rust

    for f in nc.m.functions:
        for b in f.blocks:
            il = list(b.instructions)
            need = [i for i in il
                    if i.sync_info is not None
                    and len(i.sync_info.on_wait) > max_waits]
            if not need:
                continue
            carriers = {}
            created = []
            for inst in need:
                waits = list(inst.sync_info.on_wait)
                cs = []
                for wt in waits[max_waits:]:
                    h = bass_rust.SemaphoreHandle(wt.ant_name, wt.id)
                    ev = nc.engines[inst.engine].wait_op(
                        h, wt.wait_value, _WAIT_OP_FROM_MODE[wt.wait_mode])
                    cs.append(ev.ins)
                    created.append(ev.ins)
                si = inst.sync_info
                si.on_wait = waits[:max_waits]
                inst.sync_info = si
                carriers[inst.name] = cs
            # the new instructions were appended to nc.cur_bb; remove them
            # from wherever they landed, then splice before their drains.
            created_names = {i.name for i in created}
            for f2 in nc.m.functions:
                for b2 in f2.blocks:
                    lst = list(b2.instructions)
                    kept = [i for i in lst if i.name not in created_names]
                    if len(kept) != len(lst):
                        b2.instructions = kept
            out = []
            for inst in list(b.instructions):
                out.extend(carriers.get(inst.name, []))
                out.append(inst)
            b.instructions = out

# cast_mode: 'rne' = hw f32->int32 cast rounds to nearest; 'trunc' = truncates
CAST_MODE = os.environ.get('DEFATT_CAST_MODE', 'rne')


def build_nc(n_tiles=NT_FULL, gather_bufs=4, work_bufs=3, debug=False):
    qpad = n_tiles * P
    nc = bass.Bass("TRN2")
    dbg = {}
    if debug:
        for nm, shp, dt_ in [("d_lin", [P, 24], F32), ("d_iall", [P, 16], F32),
                             ("d_x0f", [P, 16], F32), ("d_w0", [P, 16], F32),
                             ("d_w1", [P, 16], F32), ("d_U", [P, 32], F32),
                             ("d_idx", [P, 32], I32), ("d_awe", [P, 8], F32),
                             ("d_G0", [P, 16 * C], F32)]:
            dbg[nm] = nc.dram_tensor(nm, shp, dt_, kind="ExternalOutput")

    qryT = nc.dram_tensor("qryT", [C, qpad], F32, kind="ExternalInput")
    refp = nc.dram_tensor("refp", [qpad, 4], F32, kind="ExternalInput")
    val0 = nc.dram_tensor("val0", [NV, C], F32, kind="ExternalInput")
    val1 = nc.dram_tensor("val1", [NV, C], F32, kind="ExternalInput")
    wall = nc.dram_tensor("wall", [C, 24], F32, kind="ExternalInput")
    cons = nc.dram_tensor("cons", [P, NCOL_CONST], F32, kind="ExternalInput")
    out = nc.dram_tensor("out", [qpad, C], F32, kind="ExternalOutput")

    vals = [val0, val1]

    with TileContext(nc) as tc:
        with (
            tc.tile_pool(name="const", bufs=1) as cp,
            tc.tile_pool(name="work", bufs=work_bufs) as wp,
            tc.tile_pool(name="gather", bufs=gather_bufs) as gp,
            tc.tile_pool(name="psum", bufs=2, space="PSUM") as pp,
        ):
            # ---- constants, loaded once ----
            ct = cp.tile([P, NCOL_CONST], F32)
            nc.sync.dma_start(ct[:], cons[:, :])
            smat = ct[:, 0:4]            # [W, H, W, H]
            b_all = ct[:, 4:28]          # bias (b_off || b_attn) bcast
            cmax = ct[:, 28:44]          # clamp max per (l,p,xy): x->W-2, y->H-2
            cornoff = ct[:, 44:76]       # per (l,p,yc,xc): yc*W + xc

            wt = cp.tile([P, 2, 24], F32)   # W_all split into two K-chunks
            nc.sync.dma_start(
                wt[:], wall.rearrange("(h p) n -> p h n", p=P)[:, :, :])

            qryT_r = qryT.rearrange("(h p) q -> p h q", p=P)

            for t in range(n_tiles):
                # ---- load query (pre-transposed) + reference points ----
                qT = wp.tile([P, 2, P], F32, tag="qT")
                nc.sync.dma_start(qT[:], qryT_r[:, :, ts(t, P)])
                rt = wp.tile([P, 4], F32, tag="rt")
                nc.sync.dma_start(rt[:], refp[ts(t, P), :])

                # ---- linears: lin = q @ [W_off || W_attn] + b ----
                lin_ps = pp.tile([P, 24], F32)
                nc.tensor.matmul(out=lin_ps[:], lhsT=qT[:, 0, :],
                                 rhs=wt[:, 0, :], start=True, stop=False)
                nc.tensor.matmul(out=lin_ps[:], lhsT=qT[:, 1, :],
                                 rhs=wt[:, 1, :], start=False, stop=True)
                lin = wp.tile([P, 24], F32, tag="lin")
                nc.vector.tensor_add(out=lin[:], in0=lin_ps[:], in1=b_all)

                # ---- softmax numerator/denominator over the 8 attn logits ----
                aw_e = wp.tile([P, 8], F32, tag="aw_e")
                nc.scalar.activation(aw_e[:], lin[:, 16:24],
                                     mybir.ActivationFunctionType.Exp)
                ssum = wp.tile([P, 1], F32, tag="ssum")
                nc.vector.reduce_sum(out=ssum[:], in_=aw_e[:],
                                     axis=mybir.AxisListType.X)
                rinv = wp.tile([P, 1], F32, tag="rinv")
                nc.vector.reciprocal(out=rinv[:], in_=ssum[:])

                # ---- sample coords: i = ref*scale + off - 0.5 ----
                refsc = wp.tile([P, 4], F32, tag="refsc")
                nc.vector.tensor_mul(out=refsc[:], in0=rt[:], in1=smat)
                i_all = wp.tile([P, 16], F32, tag="i_all")
                for l in range(2):
                    refsc_b = refsc[:, 2 * l:2 * l + 2] \
                        .unsqueeze(1).broadcast_to([P, 4, 2])
                    nc.vector.scalar_tensor_tensor(
                        out=i_all[:, 8 * l:8 * l + 8]
                            .rearrange("p (k x) -> p k x", k=4),
                        in0=lin[:, 8 * l:8 * l + 8]
                            .rearrange("p (k x) -> p k x", k=4),
                        scalar=-0.5, in1=refsc_b,
                        op0=mybir.AluOpType.add, op1=mybir.AluOpType.add)

                # ---- low corner: x0 = clamp(round(i - 0.5), 0, {W,H}-2) ----
                t2 = wp.tile([P, 16], F32, tag="t2")
                nc.vector.tensor_scalar(
                    out=t2[:], in0=i_all[:], scalar1=-0.5, scalar2=0.0,
                    op0=mybir.AluOpType.add, op1=mybir.AluOpType.max)
                nc.vector.tensor_tensor(out=t2[:], in0=t2[:], in1=cmax,
                                        op=mybir.AluOpType.min)
                if CAST_MODE == 'trunc':
                    nc.vector.tensor_scalar_add(out=t2[:], in0=t2[:],
                                                scalar1=0.5)
                x0i = wp.tile([P, 16], I32, tag="x0i")
                nc.vector.tensor_copy(out=x0i[:], in_=t2[:])
                x0f = wp.tile([P, 16], F32, tag="x0f")
                nc.vector.tensor_copy(out=x0f[:], in_=x0i[:])

                # ---- hat weights ----
                # w0 = relu(1-|d|)   = max(min(1-d, 1+d), 0)
                # w1 = relu(1-|d-1|) = max(min(2-d, d), 0)
                d0 = wp.tile([P, 16], F32, tag="d0")
                nc.vector.tensor_sub(out=d0[:], in0=i_all[:], in1=x0f[:])
                f0 = wp.tile([P, 16], F32, tag="f0")
                nc.vector.tensor_scalar(
                    out=f0[:], in0=d0[:], scalar1=-1.0, scalar2=1.0,
                    op0=mybir.AluOpType.mult, op1=mybir.AluOpType.add)  # 1-d
                w0 = wp.tile([P, 16], F32, tag="w0")
                nc.vector.scalar_tensor_tensor(
                    out=w0[:], in0=d0[:], scalar=1.0, in1=f0[:],
                    op0=mybir.AluOpType.add, op1=mybir.AluOpType.min)  # min(d+1,1-d)
                nc.vector.tensor_scalar_max(out=w0[:], in0=w0[:], scalar1=0.0)
                e1 = wp.tile([P, 16], F32, tag="e1")
                nc.vector.tensor_scalar(
                    out=e1[:], in0=d0[:], scalar1=-1.0, scalar2=2.0,
                    op0=mybir.AluOpType.mult, op1=mybir.AluOpType.add)  # 2-d
                w1 = wp.tile([P, 16], F32, tag="w1")
                nc.vector.scalar_tensor_tensor(
                    out=w1[:], in0=d0[:], scalar=0.0, in1=e1[:],
                    op0=mybir.AluOpType.bypass, op1=mybir.AluOpType.min)
                nc.vector.tensor_scalar_max(out=w1[:], in0=w1[:], scalar1=0.0)

                # ---- combine weights: u = aw * wy * wx for 4 corners ----
                U = wp.tile([P, 4, 8], F32, tag="U")
                t0 = wp.tile([P, 8], F32, tag="t0")
                nc.vector.scalar_tensor_tensor(
                    out=t0[:], in0=aw_e[:], scalar=rinv[:, 0:1],
                    in1=w0[:, 1:16:2],
                    op0=mybir.AluOpType.mult, op1=mybir.AluOpType.mult)
                t1 = wp.tile([P, 8], F32, tag="t1")
                nc.vector.scalar_tensor_tensor(
                    out=t1[:], in0=aw_e[:], scalar=rinv[:, 0:1],
                    in1=w1[:, 1:16:2],
                    op0=mybir.AluOpType.mult, op1=mybir.AluOpType.mult)
                nc.vector.tensor_mul(out=U[:, 0, :], in0=t0[:], in1=w0[:, 0:16:2])
                nc.vector.tensor_mul(out=U[:, 1, :], in0=t0[:], in1=w1[:, 0:16:2])
                nc.vector.tensor_mul(out=U[:, 2, :], in0=t1[:], in1=w0[:, 0:16:2])
                nc.vector.tensor_mul(out=U[:, 3, :], in0=t1[:], in1=w1[:, 0:16:2])

                # ---- corner row indices ----
                idx00 = wp.tile([P, 8], F32, tag="idx00")
                nc.vector.scalar_tensor_tensor(
                    out=idx00[:], in0=x0f[:, 1:16:2], scalar=float(W),
                    in1=x0f[:, 0:16:2],
                    op0=mybir.AluOpType.mult, op1=mybir.AluOpType.add)
                idxf = wp.tile([P, 32], F32, tag="idxf")
                nc.vector.scalar_tensor_tensor(
                    out=idxf[:].rearrange("p (j k) -> p j k", j=8),
                    in0=idx00[:].unsqueeze(2).broadcast_to([P, 8, 4]),
                    scalar=0.0,
                    in1=cornoff.rearrange("p (j k) -> p j k", j=8),
                    op0=mybir.AluOpType.bypass, op1=mybir.AluOpType.add)
                idxi = wp.tile([P, 32], I32, tag="idxi")
                nc.vector.tensor_copy(out=idxi[:], in_=idxf[:])

                # ---- gather: one indirect DMA per corner ([128,1] offsets —
                # the only offset pattern this HW's SWDGE honors) ----
                G = [None, None]
                for l in range(2):
                    G[l] = gp.tile([P, 16, C], F32, tag="G", name=f"G{l}_{t}")
                    for j in range(16):
                        col = l * 16 + j
                        nc.gpsimd.indirect_dma_start(
                            out=G[l][:, j, :], out_offset=None,
                            in_=vals[l][:, :],
                            in_offset=bass.IndirectOffsetOnAxis(
                                ap=idxi[:, col:col + 1], axis=0),
                        )

                # ---- weighted combine: 32 fused MACs ----
                acc = wp.tile([P, C], F32, tag="acc")
                first = True
                for l in range(2):
                    for p4 in range(4):
                        for k in range(4):       # (yc,xc) = k//2, k%2
                            j = p4 * 4 + k
                            u_sc = U[:, k, l * 4 + p4:l * 4 + p4 + 1]
                            if first:
                                nc.vector.tensor_scalar_mul(
                                    out=acc[:], in0=G[l][:, j, :], scalar1=u_sc)
                                first = False
                            else:
                                nc.vector.scalar_tensor_tensor(
                                    out=acc[:], in0=G[l][:, j, :], scalar=u_sc,
                                    in1=acc[:], op0=mybir.AluOpType.mult,
                                    op1=mybir.AluOpType.add)

                if debug and t == 0:
                    nc.sync.dma_start(dbg["d_lin"][:, :], lin[:])
                    nc.sync.dma_start(dbg["d_iall"][:, :], i_all[:])
                    nc.sync.dma_start(dbg["d_x0f"][:, :], x0f[:])
                    nc.sync.dma_start(dbg["d_w0"][:, :], w0[:])
                    nc.sync.dma_start(dbg["d_w1"][:, :], w1[:])
                    nc.sync.dma_start(
                        dbg["d_U"][:, :],
                        U[:].rearrange("p a b -> p (a b)"))
                    nc.sync.dma_start(dbg["d_idx"][:, :], idxi[:])
                    nc.sync.dma_start(dbg["d_awe"][:, :], aw_e[:])
                    nc.sync.dma_start(
                        dbg["d_G0"][:, :],
                        G[0][:].rearrange("p a b -> p (a b)"))

                nc.sync.dma_start(out[ts(t, P), :], acc[:])

    _split_multiwait_noctrl(nc)
    return nc


def make_consts():
    """Host-side constant rows. ref cols are (l0x, l0y, l1x, l1y)."""
    smat = np.array([W, H, W, H], np.float32)
    cmax = np.tile(np.array([W - 2, H - 2], np.float32), 8)
    cornoff = np.tile(np.array([0, 1, W, W + 1], np.float32), 8)
    return smat, cmax, cornoff


def pack_consts(b_off, b_attn):
    smat, cmax, cornoff = make_consts()
    b_all = np.concatenate([b_off, b_attn]).astype(np.float32)
    row = np.concatenate([smat, b_all, cmax, cornoff])
    assert row.shape[0] == NCOL_CONST
    return np.ascontiguousarray(np.broadcast_to(row, (P, NCOL_CONST)))


_CACHED = {}


def _get_nc():
    if 'nc' not in _CACHED:
        _CACHED['nc'] = build_nc()
    return _CACHED['nc']


def kernel(query, key, value, reference_points, spatial_shapes,
           W_off, b_off, W_attn, b_attn):
    from concourse import bass_utils

    query = np.asarray(query, np.float32)
    value = np.asarray(value, np.float32)
    reference_points = np.asarray(reference_points, np.float32)
    W_off = np.asarray(W_off, np.float32)
    W_attn = np.asarray(W_attn, np.float32)
    b_off = np.asarray(b_off, np.float32)
    b_attn = np.asarray(b_attn, np.float32)

    nc = _get_nc()

    wall = np.ascontiguousarray(np.concatenate([W_off, W_attn], axis=1))
    cons = pack_consts(b_off, b_attn)
    val0 = np.ascontiguousarray(value[0])
    val1 = np.ascontiguousarray(value[1])

    q = query[0]                       # [Q, C]
    rp = reference_points[0].reshape(QTOT, 4)

    in_maps = []
    for c in range(N_CORES):
        sl = slice(c * QPC, (c + 1) * QPC)
        qc = q[sl]
        rc = rp[sl]
        pad = QPAD - QPC
        qc = np.concatenate([qc, np.broadcast_to(qc[-1:], (pad, C))], 0)
        rc = np.concatenate([rc, np.broadcast_to(rc[-1:], (pad, 4))], 0)
        in_maps.append({
            "qryT": np.ascontiguousarray(qc.T),
            "refp": np.ascontiguousarray(rc),
            "val0": val0,
            "val1": val1,
            "wall": wall,
            "cons": cons,
        })

    _CACHED['in_maps'] = in_maps
    res = bass_utils.run_bass_kernel_spmd(nc, in_maps,
                                          core_ids=list(range(N_CORES)))
    outs = [r["out"][:QPC] for r in res.results]
    full = np.concatenate(outs, axis=0)[None]     # [1, 40000, 256]
    return full.astype(np.float32)

